# revision 1
# baseline (speedup 1.0000x reference)
"""Trainium2 Bass kernel for causal MultiHeadAttention (B=4,S=2048,E=1024,H=16).

Sharding: 8 cores = (batch b, head-half) grid. Core c handles batch c//2 and
heads [8*(c%2), 8*(c%2)+8). Each core computes its 8 heads' attention and the
partial output projection (its 512 rows of Wo); the host sums the two partials
per batch and adds the bias (the 2-way "all-reduce" done at unshard time).

On-core dataflow (bf16 matmul operands, fp32 PSUM accumulation):
  P2: QT/KT stored head-pair-packed [64*h0 | 64*h1] on the partition dim
      (no zero padding); V natural [s, 8*dh] per-head tiles [V | ones | pad].
  P3: scores via 2x ROW-TILED matmuls (K=64 per head, tile_position rows 0/64
      auto-derived from base partitions) - both heads of a pair run
      concurrently in the PE array. exp on ACT (scale fused). Causal handled
      by: (a) ragged score/PV matmuls on diagonal-block tiles (skip fully
      masked q columns), (b) one [128,128] triangular mask multiply per
      diagonal tile. Softmax denominator = ones-column of V via the PV
      matmul's row 64. PV accumulation lags two units behind the score
      stream (one near the end to shorten the tail); projections for the
      next head-pair are interleaved between units so ACT (the P3
      bottleneck) starts as early as possible and PE always has fill work.
  P4: output projection from outT [concat-head-dim, s] x Wo rows; 4 s-tiles
      per output DMA (dma_start dispatch costs ~600ns on the queue).
All input tensors are shipped in tile-ready layouts so each SBUF tile is a
single dma_start.
"""

import sys

if "/opt/trn_rl_repo" not in sys.path:
    sys.path.insert(0, "/opt/trn_rl_repo")

import numpy as np
from contextlib import ExitStack

B, S, E, H = 4, 2048, 1024, 16
DH = E // H          # 64
NCORES = 8
NH = 8               # local heads per core
HP = NH // 2         # head pairs
P = 128
NE = E // P          # 8 e-tiles
NT = S // P          # 16 s/t tiles
CH = 512
NCH = S // CH        # 4 q-chunks
SCALE = 1.0 / 8.0    # 1/sqrt(DH)

_CACHE = {}


def _build_nc():
    import concourse.mybir as mybir
    import concourse.tile as tile
    import concourse.bass as bass
    from concourse import bacc

    f32 = mybir.dt.float32
    bf16 = mybir.dt.bfloat16
    Exp = mybir.ActivationFunctionType.Exp
    PSUM = bass.MemorySpace.PSUM

    nc = bacc.Bacc(None)
    # x pre-transposed, split into two chunk-pair halves: [cp, e, 1024]
    x_d = nc.dram_tensor("x", [2, E, S // 2], bf16, kind="ExternalInput")
    # wq/wk: [p, hp, et, m] = W[et*128+p, hp*128+m]
    wq_d = nc.dram_tensor("wq", [P, HP, NE, P], bf16, kind="ExternalInput")
    wk_d = nc.dram_tensor("wk", [P, HP, NE, P], bf16, kind="ExternalInput")
    # wv: [p, et, n] = Wv[et*128+p, n]
    wv_d = nc.dram_tensor("wv", [P, NE, NH * DH], bf16, kind="ExternalInput")
    # wo: [p, ech, hp, c] = Wo[hp*128+p, ech*512+c]
    wo_d = nc.dram_tensor("wo", [P, E // CH, HP, CH], bf16, kind="ExternalInput")
    tri_d = nc.dram_tensor("tri", [P, P], bf16, kind="ExternalInput")
    out_d = nc.dram_tensor("out", [S, E], f32, kind="ExternalOutput")

    with ExitStack() as ctx:
        tc = ctx.enter_context(tile.TileContext(nc))
        persist = ctx.enter_context(tc.tile_pool(name="persist", bufs=1))
        # head-pair-packed: rows 0:64 = even head, 64:128 = odd head
        qt = persist.tile([P, HP, S], bf16)
        kt = persist.tile([P, HP, S], bf16)
        vf = persist.tile([P, NT, NH, P], bf16)       # V | ones | pad(0)
        tri = persist.tile([P, P], bf16)
        nc.sync.dma_start(out=tri, in_=tri_d[:])
        nc.vector.memset(vf.rearrange("p a b c -> p (a b c)"), 0.0)
        nc.vector.memset(vf[:, :, :, DH:DH + 1], 1.0)

        wqk = ctx.enter_context(tc.tile_pool(name="wqk", bufs=1))
        otp = ctx.enter_context(tc.tile_pool(name="otp", bufs=1))
        ptp = ctx.enter_context(tc.tile_pool(name="ptp", bufs=20))
        pvo = ctx.enter_context(tc.tile_pool(name="pvo", bufs=6))
        dnp = ctx.enter_context(tc.tile_pool(name="dnp", bufs=2))
        bcp = ctx.enter_context(tc.tile_pool(name="bcp", bufs=3))
        drp = ctx.enter_context(tc.tile_pool(name="drp", bufs=3, space="DRAM"))
        # PSUM: sp 3x2 banks + 2 shared proj/PV banks = 8 exactly
        psA = ctx.enter_context(tc.tile_pool(name="psA", bufs=3, space=PSUM))
        psB = ctx.enter_context(tc.tile_pool(name="psB", bufs=2, space=PSUM))
        # explicitly scoped (LIFO) so they can be popped mid-kernel: xtp
        # after the last projection chunk, wvp (on top) after the V phase
        xtp_cm = tc.tile_pool(name="xtp", bufs=1)
        xtp = xtp_cm.__enter__()
        wvp_cm = tc.tile_pool(name="wvp", bufs=1)
        wvp = wvp_cm.__enter__()

        # ---- input DMA: one dma_start per tile (dispatch ~600ns each).
        # x first half, wv, x second half on the SP queue (the first half
        # unblocks QK chunks 0-1 and V s-tiles 0-7); wq/wk (head-pair 0
        # first) + wo on the ACT queue.
        xts = [xtp.tile([P, S], bf16, tag=f"xt{et}", name="xt")
               for et in range(NE)]
        HS = S // 2
        for et in range(NE):
            nc.sync.dma_start(out=xts[et][:, 0:HS],
                              in_=x_d[0, et * P:(et + 1) * P, :])
        wv = wvp.tile([P, NE, NH * DH], bf16)
        nc.sync.dma_start(out=wv, in_=wv_d[:])
        for et in range(NE):
            nc.sync.dma_start(out=xts[et][:, HS:S],
                              in_=x_d[1, et * P:(et + 1) * P, :])

        wts = {}
        for hp in range(HP):
            for wi, wd in enumerate((wq_d, wk_d)):
                wt = wqk.tile([P, NE, P], bf16, tag=f"wt{hp}{wi}", name="wt")
                nc.scalar.dma_start(out=wt, in_=wd[:, hp])
                wts[(hp, wi)] = wt

        wt2s = []
        for ech in range(E // CH):
            wt2 = otp.tile([P, HP, CH], bf16, tag=f"wt2{ech}", name="wt2")
            nc.scalar.dma_start(out=wt2, in_=wo_d[:, ech])
            wt2s.append(wt2)

        outTs = [otp.tile([P, S], bf16, tag=f"outT{i}", name="outT")
                 for i in range(HP)]

        # ---- P2 emission helpers (interleaved into the P3 unit stream) ----
        def emit_qk_chunk(hp, chk, et_outer=False):
            cs = slice(chk * CH, (chk + 1) * CH)
            if et_outer:
                # prologue form: start as soon as the first x tile lands
                pss = {}
                for wi in range(2):
                    pss[wi] = psB.tile([P, CH], f32, tag="w", name="w")
                for et in range(NE):
                    for wi in range(2):
                        nc.tensor.matmul(
                            pss[wi], wts[(hp, wi)][:, et, :], xts[et][:, cs],
                            start=(et == 0), stop=(et == NE - 1))
                for wi, dst in ((0, qt), (1, kt)):
                    nc.vector.tensor_copy(out=dst[:, hp, cs], in_=pss[wi])
            else:
                for wi, dst in ((0, qt), (1, kt)):
                    ps = psB.tile([P, CH], f32, tag="w", name="w")
                    for et in range(NE):
                        nc.tensor.matmul(
                            ps, wts[(hp, wi)][:, et, :], xts[et][:, cs],
                            start=(et == 0), stop=(et == NE - 1))
                    nc.vector.tensor_copy(out=dst[:, hp, cs], in_=ps)

        def emit_v_group(grp):
            """V natural for s-tiles 4*grp..4*grp+4, all 8 heads."""
            for st in range(4 * grp, 4 * grp + 4):
                ps = psB.tile([P, NH * DH], f32, tag="w", name="w")
                for et in range(NE):
                    nc.tensor.matmul(
                        ps, xts[et][:, st * P:(st + 1) * P], wv[:, et, :],
                        start=(et == 0), stop=(et == NE - 1))
                nc.vector.tensor_copy(
                    out=vf[:, st, :, 0:DH],
                    in_=ps.rearrange("p (h d) -> p h d", h=NH))

        # ---- P3: attention units ----

        def emit_unit(hp, chk):
            """Row-tiled scores + exp + triangular mask for (hp, chk).
            Returns pts: {h: [pt pair tiles]}."""
            ntv = 4 * chk + 4
            nprs = ntv // 2
            pts = {0: [], 1: []}
            for pr in range(nprs):
                sps = {}
                qlos = []
                for j in range(2):
                    tt = 2 * pr + j
                    r = tt - 4 * chk
                    qlo = 128 * r if r > 0 else 0
                    qlos.append(qlo)
                    for h in range(2):
                        if h not in sps:
                            sps[h] = psA.tile(
                                [P, 2 * CH], f32, tag="sp", name="sp")
                        nc.tensor.matmul(
                            sps[h][:, j * CH + qlo:(j + 1) * CH],
                            kt[h * DH:(h + 1) * DH, hp, tt * P:(tt + 1) * P],
                            qt[h * DH:(h + 1) * DH, hp,
                               chk * CH + qlo:(chk + 1) * CH],
                            start=True, stop=True)
                diag = (2 * pr - 4 * chk) >= 0
                for h in range(2):
                    pt = ptp.tile([P, 2 * CH], bf16, tag="pt", name="pt")
                    if diag:
                        for j in range(2):
                            qlo = qlos[j]
                            cs = slice(j * CH + qlo, (j + 1) * CH)
                            nc.scalar.activation(
                                out=pt[:, cs], in_=sps[h][:, cs],
                                func=Exp, scale=SCALE)
                        # triangular mask on each diagonal 128-block
                        for j in range(2):
                            r = 2 * pr + j - 4 * chk
                            ms = slice(j * CH + 128 * r, j * CH + 128 * r + P)
                            nc.vector.tensor_mul(pt[:, ms], pt[:, ms], tri)
                    else:
                        nc.scalar.activation(
                            out=pt, in_=sps[h], func=Exp, scale=SCALE)
                    pts[h].append(pt)
            return pts

        def emit_pv(hp, chk, pts):
            """PV accumulation for a completed unit (ragged on diag tiles)."""
            ntv = 4 * chk + 4
            po = pvo.tile([P, CH], bf16, tag="po", name="po")
            dd2 = drp.tile([2, CH], f32, tag="dd", name="dd")
            for h in range(2):
                pv = psB.tile([P, CH], f32, tag="w", name="w")
                for tt in range(ntv):
                    r = tt - 4 * chk
                    qlo = 128 * r if r > 0 else 0
                    nc.tensor.matmul(
                        pv[:, qlo:CH],
                        vf[:, tt, 2 * hp + h, :],
                        pts[h][tt // 2][:, (tt % 2) * CH + qlo:
                                        (tt % 2 + 1) * CH],
                        start=(tt == 0), stop=(tt == ntv - 1),
                        skip_group_check=True)
                nc.vector.tensor_copy(
                    out=po[h * DH:(h + 1) * DH, :], in_=pv[0:DH, :])
                den = dnp.tile([1, CH], f32, tag="den", name="den")
                nc.vector.tensor_copy(out=den, in_=pv[DH:DH + 1, :])
                rdn = dnp.tile([1, CH], f32, tag="rdn", name="rdn")
                nc.vector.reciprocal_approx_fast(out=rdn, in_=den)
                nc.scalar.dma_start(out=dd2[h:h + 1, :], in_=rdn)
            bc = bcp.tile([P, CH], f32, tag="bc", name="bc")
            for h in range(2):
                row = dd2[h:h + 1, :]
                src = bass.AP(
                    tensor=row.tensor, offset=row.offset,
                    ap=[[0, DH]] + list(row.ap[1:]))
                nc.sync.dma_start(
                    out=bc[h * DH:(h + 1) * DH, :], in_=src)
            cs = slice(chk * CH, (chk + 1) * CH)
            nc.gpsimd.tensor_mul(outTs[hp][:, cs], po, bc)

        # ---- emission schedule ----
        # prologue: chunks 0 and 1 of head-pair 0, paced by the first-half
        # x DMA stream (et-outer: starts on the first landed tile)
        emit_qk_chunk(0, 0, et_outer=True)
        emit_qk_chunk(0, 1, et_outer=True)

        from collections import deque
        pend_q = deque()
        # remaining projection chunks, spread one per slot with a 2-slot
        # just-in-time lead so hp3's slots also have PE fill work
        qk_queue = deque((h2, c2) for h2 in range(1, HP) for c2 in range(NCH))
        for hp in range(HP):
            for chk in range(NCH):
                pts = emit_unit(hp, chk)
                pend_q.append((hp, chk, pts))
                if hp == 0:
                    emit_v_group(chk)              # V s-tiles for everyone
                    if chk < NCH - 2:
                        emit_qk_chunk(0, chk + 2)  # own remaining chunks
                if hp * NCH + chk >= 2 and qk_queue:
                    emit_qk_chunk(*qk_queue.popleft())
                if len(pend_q) > 2:
                    emit_pv(*pend_q.popleft())
                if hp == 0 and chk == NCH - 1:
                    wvp_cm.__exit__(None, None, None)  # wv tiles done
                if hp == HP - 1 and chk == 1:
                    xtp_cm.__exit__(None, None, None)  # x tiles done
        # ---- P4: output projection (partial: local 512 rows of Wo) ----
        osb = ctx.enter_context(tc.tile_pool(name="osb", bufs=3))

        def emit_p4_group(ech, st4):
            ob = osb.tile([P, 4, CH], f32, tag="ob", name="ob")
            for k in range(4):
                st = st4 * 4 + k
                ps = psB.tile([P, CH], f32, tag="w", name="w")
                for hp in range(HP):
                    nc.tensor.matmul(
                        ps, outTs[hp][:, st * P:(st + 1) * P],
                        wt2s[ech][:, hp, :],
                        start=(hp == 0), stop=(hp == HP - 1))
                # ACT is idle in the tail - split evacuation work
                if k % 2 == 0:
                    nc.scalar.copy(out=ob[:, k, :], in_=ps)
                else:
                    nc.vector.tensor_copy(out=ob[:, k, :], in_=ps)
            # one DMA per 4 s-tiles
            dst = out_d[st4 * 4 * P:(st4 + 1) * 4 * P,
                        ech * CH:(ech + 1) * CH]
            srcap = bass.AP(
                tensor=dst.tensor, offset=dst.offset,
                ap=[[dst.ap[0][0], P], [P * dst.ap[0][0], 4],
                    list(dst.ap[1])])
            nc.sync.dma_start(out=srcap, in_=ob)

        # flush the PV pipeline with the first P4 groups interleaved as
        # PE fill work (their outT chunk deps are complete by now)
        emit_pv(*pend_q.popleft())
        emit_p4_group(0, 0)
        emit_pv(*pend_q.popleft())
        assert not pend_q
        for st4 in range(1, NT // 4):
            emit_p4_group(0, st4)
        for st4 in range(NT // 4):
            emit_p4_group(1, st4)

    nc.finalize()
    return nc


def _get_nc():
    if "nc" not in _CACHE:
        _CACHE["nc"] = _build_nc()
    return _CACHE["nc"]


def _make_in_maps(x, Wq, Wk, Wv, Wo):
    import ml_dtypes

    bf = ml_dtypes.bfloat16
    pcol = np.arange(P)[:, None]
    frow = np.arange(P)[None, :]
    tri = (pcol <= frow).astype(bf)
    in_maps = []
    for c in range(NCORES):
        b, half = divmod(c, 2)
        hs = slice(half * NH, (half + 1) * NH)
        wq = Wq[hs].transpose(1, 0, 2).reshape(E, NH * DH)
        wk = Wk[hs].transpose(1, 0, 2).reshape(E, NH * DH)
        wv = Wv[hs].transpose(1, 0, 2).reshape(E, NH * DH)
        wo = Wo[half * NH * DH:(half + 1) * NH * DH]  # [512, E]
        in_maps.append({
            "x": np.ascontiguousarray(
                x[b].T.reshape(E, 2, S // 2).transpose(1, 0, 2).astype(bf)),
            "wq": np.ascontiguousarray(
                wq.reshape(NE, P, HP, P).transpose(1, 2, 0, 3).astype(bf)),
            "wk": np.ascontiguousarray(
                wk.reshape(NE, P, HP, P).transpose(1, 2, 0, 3).astype(bf)),
            "wv": np.ascontiguousarray(
                wv.reshape(NE, P, NH * DH).transpose(1, 0, 2).astype(bf)),
            "wo": np.ascontiguousarray(
                wo.reshape(HP, P, E // CH, CH).transpose(1, 2, 0, 3)
                .astype(bf)),
            "tri": tri,
        })
    return in_maps


def _ensure_ntff_hook():
    """Register the axon NTFF profile hook under antenv.axon_hooks.

    The agent image's antenv lacks the axon_hooks module, so
    run_bass_kernel_spmd(trace=True) would silently skip profiling.
    Recreate the module in sys.modules using trn_agent_boot's ctypes hook.
    """
    import types
    try:
        import antenv.axon_hooks  # noqa: F401
        return
    except ImportError:
        pass
    try:
        from trn_agent_boot.trn_boot import _ntff_profile_via_ctypes
        hook = _ntff_profile_via_ctypes("/opt/axon/libaxon_pjrt.so")
    except Exception:
        hook = None
    mod = types.ModuleType("antenv.axon_hooks")
    mod.get_axon_ntff_profile_hook = lambda: hook
    mod.set_axon_ntff_profile_hook = lambda h: None
    sys.modules["antenv.axon_hooks"] = mod


def _run(inputs, trace=False):
    from concourse.bass_utils import run_bass_kernel_spmd

    if trace:
        _ensure_ntff_hook()

    x = np.asarray(inputs["x"], dtype=np.float32)
    Wq = np.asarray(inputs["Wq"], dtype=np.float32)
    Wk = np.asarray(inputs["Wk"], dtype=np.float32)
    Wv = np.asarray(inputs["Wv"], dtype=np.float32)
    Wo = np.asarray(inputs["Wo"], dtype=np.float32)
    bo = np.asarray(inputs["bo"], dtype=np.float32)

    nc = _get_nc()
    in_maps = _make_in_maps(x, Wq, Wk, Wv, Wo)
    res = run_bass_kernel_spmd(nc, in_maps, list(range(NCORES)), trace=trace)
    out = np.empty((B, S, E), dtype=np.float32)
    for b in range(B):
        out[b] = res.results[2 * b]["out"] + res.results[2 * b + 1]["out"] + bo
    return out, res


def kernel(**inputs):
    out, _ = _run(inputs, trace=False)
    return out

